# revision 2
# baseline (speedup 1.0000x reference)
"""BitNet MLP (SwiGLU, ternary weights) on 8 Trainium2 NeuronCores.

Strategy: 8-way data-parallel over the 4096 tokens (512 tokens/core),
weights replicated. No collectives. Everything is laid out
[feature, token] (transposed host-side) so every matmul has the
contraction dim on partitions and needs no on-device transposes:

  phase 2: gate/up projections + SwiGLU
      psum[i_tile(128), T=512] += wgu_tile[kp=128, 128].T @ xt[kp=128, T]
      inter = silu(gate*gs) * up                      (kept resident, bf16)
  phase 3: down projection
      psum[h_tile(128), T] += wd_tile[ip=128, 128].T @ inter[ip=128, T]
      out = psum * (us*ds)

All matmuls are bf16 (ternary weights are exact in bf16), N=512 = one
PSUM bank, K=128. Weights stream from HBM in 2 MiB blocks, alternating
between the two HWDGE rings (sync + scalar) so each ring runs at ~75
GB/s instead of one at 150. ~30 warm-up matmuls on a memset tile run
during the DMA lead-in so the PE's HAM clock-gate reaches 8/8 before
the first real matmul.
"""

import sys

for _p in ("/opt/trn_rl_repo",):
    if _p not in sys.path:
        sys.path.append(_p)

import numpy as np
import ml_dtypes

import concourse.bacc as bacc
import concourse.mybir as mybir
import concourse.tile as tile
from concourse.bass_utils import run_bass_kernel_spmd

BF16 = ml_dtypes.bfloat16

N_CORES = 8
H = 4096          # hidden
I = 11008         # intermediate
T = 512           # tokens per core (4096 / 8)
P = 128
KO = H // P       # 32 k-tiles for gate/up contraction
IT = I // P       # 86 i-tiles
NG2 = IT // 2     # 43 phase-2 groups (2 i-tiles each)
KH = 2            # k halves (16 k-tiles per DMA block)
HG = (H // P) // 4  # 8 phase-3 groups (4 h-tiles each)
IB = 6            # phase-3 i-blocks of 16 (86 padded to 96)
IT_PAD = IB * 16
N_WARM = 30       # PE warm-up matmuls (N=128 each) during DMA lead-in

_CACHE = {}


def _build_nc(gate_scale: float, up_scale: float, down_scale: float):
    nc = bacc.Bacc("TRN2", target_bir_lowering=False, debug=False,
                   enable_asserts=False, num_devices=N_CORES)
    f32 = mybir.dt.float32
    bf16 = mybir.dt.bfloat16

    xt_d = nc.dram_tensor("xt", [P, KO, T], bf16, kind="ExternalInput")
    wgu_d = nc.dram_tensor("wgu", [NG2, KH, P, 16, 512], bf16, kind="ExternalInput")
    wd_d = nc.dram_tensor("wd", [HG, IB, P, 16, 512], bf16, kind="ExternalInput")
    out_d = nc.dram_tensor("out", [HG, P, 4, T], f32, kind="ExternalOutput")

    blk_sizes = [16] * (IB - 1) + [IT - 16 * (IB - 1)]  # [16,16,16,16,16,6]

    with tile.TileContext(nc) as tc:
        with (
            tc.tile_pool(name="xpool", bufs=1) as xpool,
            tc.tile_pool(name="wpool", bufs=3) as wpool,
            tc.tile_pool(name="ipool", bufs=1) as ipool,
            tc.tile_pool(name="tpool", bufs=2) as tpool,
            tc.tile_pool(name="opool", bufs=2) as opool,
            tc.tile_pool(name="dpool", bufs=1) as dpool,
            tc.tile_pool(name="psum", bufs=2, space="PSUM") as psum,
        ):
            # ---- phase 0: PE warm-up ----
            # The HAM clock gate keeps the PE at 1.2 GHz until it has seen
            # ~3.4us of sustained matmul activity.  Spend the DMA lead-in
            # warming it on a memset tile so real matmuls run at 2.4 GHz
            # from the first one.
            dummy = dpool.tile([P, P], bf16)
            nc.vector.memset(dummy[:], 0)
            warm_ps = psum.tile([P, T], f32, tag="p0")
            for _ in range(N_WARM):
                nc.tensor.matmul(warm_ps[:, 0:P], dummy[:], dummy[:],
                                 start=True, stop=True)

            # xt goes on the scalar HWDGE ring (parallel to weight traffic
            # on the sync ring), in escalating k-slice chunks so the first
            # matmuls don't wait for the whole 4 MiB.
            xt_sb = xpool.tile([P, KO, T], bf16)
            for lo, hi in ((0, 1), (1, 2), (2, 4), (4, 8), (8, 16), (16, 24), (24, 32)):
                nc.scalar.dma_start(xt_sb[:, lo:hi, :], xt_d.ap()[:, lo:hi, :])
            inter_sb = ipool.tile([P, IT, T], bf16)

            # ---- phase 2: gate/up matmuls + SwiGLU ----
            for g in range(NG2):
                pg0 = psum.tile([P, T], f32, tag="p0")
                pg1 = psum.tile([P, T], f32, tag="p1")
                pu0 = psum.tile([P, T], f32, tag="p2")
                pu1 = psum.tile([P, T], f32, tag="p3")
                for kh in range(KH):
                    w = wpool.tile([P, 16, 512], bf16, tag="w")
                    if g == 0 and kh == 0:
                        # escalating first blocks so MM 0 starts early
                        for lo, hi in ((0, 1), (1, 2), (2, 4), (4, 8), (8, 16)):
                            nc.sync.dma_start(w[:, lo:hi, :],
                                              wgu_d.ap()[g, kh, :, lo:hi, :])
                    elif g == 0:
                        for wc in range(4):
                            nc.sync.dma_start(w[:, 4 * wc:4 * (wc + 1), :],
                                              wgu_d.ap()[g, kh, :, 4 * wc:4 * (wc + 1), :])
                    else:
                        # alternate HWDGE rings: kh=0 on sync, kh=1 on scalar
                        eng = nc.sync if kh == 0 else nc.scalar
                        eng.dma_start(w[:], wgu_d.ap()[g, kh])
                    for ko in range(16):
                        k = kh * 16 + ko
                        rhs = xt_sb[:, k, :]
                        st, sp = k == 0, k == KO - 1
                        nc.tensor.matmul(pg0[:], w[:, ko, 0:128], rhs, start=st, stop=sp)
                        nc.tensor.matmul(pg1[:], w[:, ko, 128:256], rhs, start=st, stop=sp)
                        nc.tensor.matmul(pu0[:], w[:, ko, 256:384], rhs, start=st, stop=sp)
                        nc.tensor.matmul(pu1[:], w[:, ko, 384:512], rhs, start=st, stop=sp)
                for j, (pg, pu) in enumerate(((pg0, pu0), (pg1, pu1))):
                    silu_t = tpool.tile([P, T], bf16, tag="silu")
                    nc.scalar.activation(silu_t[:], pg[:],
                                         mybir.ActivationFunctionType.Silu,
                                         scale=gate_scale)
                    nc.vector.tensor_mul(inter_sb[:, 2 * g + j, :], silu_t[:], pu[:])

            # ---- phase 3: down matmul ----
            out_scale = up_scale * down_scale
            for hg in range(HG):
                pd = [psum.tile([P, T], f32, tag=f"p{j}", name=f"pd{j}") for j in range(4)]
                for b in range(IB):
                    wd_sb = wpool.tile([P, 16, 512], bf16, tag="w")
                    eng = nc.sync if b % 2 == 0 else nc.scalar
                    eng.dma_start(wd_sb[:], wd_d.ap()[hg, b])
                    # very last block of the kernel: stagger so psums j0/j1
                    # finish ~1.3 us before j2/j3 and their epilogue + out-DMA
                    # drain while j2/j3 still compute (shortens the tail)
                    stagger = (hg == HG - 1) and (b == IB - 1)
                    for io in range(blk_sizes[b]):
                        i = b * 16 + io
                        if stagger and i >= IT - 3:
                            continue
                        rhs = inter_sb[:, i, :]
                        st, sp = i == 0, i == IT - 1
                        for j in range(4):
                            nc.tensor.matmul(pd[j][:], wd_sb[:, io, j * 128:(j + 1) * 128],
                                             rhs, start=st, stop=sp)
                    if stagger:
                        for jj in ((0, 1), (2, 3)):
                            for i in range(IT - 3, IT):
                                io = i - b * 16
                                rhs = inter_sb[:, i, :]
                                for j in jj:
                                    nc.tensor.matmul(pd[j][:],
                                                     wd_sb[:, io, j * 128:(j + 1) * 128],
                                                     rhs, start=False, stop=(i == IT - 1))
                ob = opool.tile([P, 4, T], f32, tag="ob")
                for j in range(4):
                    # alternate ACT/DVE so the final scale-copies don't
                    # serialize on one engine queue behind the last matmul
                    if j % 2 == 0:
                        nc.scalar.activation(ob[:, j, :], pd[j][:],
                                             mybir.ActivationFunctionType.Copy,
                                             scale=out_scale)
                    else:
                        nc.vector.tensor_scalar_mul(ob[:, j, :], pd[j][:], out_scale)
                    # sync queue is idle at the kernel tail; scalar is not
                    nc.sync.dma_start(out_d.ap()[hg, :, j, :], ob[:, j, :])

    nc.compile()
    return nc


def _pack_weights(gate_w, up_w, down_w):
    # wgu[g, kh, p, ko, c]: c 0:128 -> gate i-tile 2g, 128:256 -> gate 2g+1,
    # 256:384 -> up 2g, 384:512 -> up 2g+1. p = k_p (contraction partition).
    def pack2(wm):
        a = np.ascontiguousarray(wm, dtype=np.float32).reshape(NG2, 256, KH, 16, P)
        return a.transpose(0, 2, 4, 3, 1)  # [g, kh, p, ko, m]

    gt = pack2(gate_w)
    ut = pack2(up_w)
    wgu = np.concatenate([gt, ut], axis=-1).astype(BF16)
    wgu = np.ascontiguousarray(wgu)

    # wd[hg, b, p, io, j*128+h_p] = down_w[(hg*4+j)*128+h_p, (b*16+io)*128+p]
    dp = np.zeros((H, IT_PAD * P), dtype=np.float32)
    dp[:, :I] = down_w
    d = dp.reshape(HG, 4, P, IB, 16, P)          # [hg, j, h_p, b, io, p]
    wd = d.transpose(0, 3, 5, 4, 1, 2)           # [hg, b, p, io, j, h_p]
    wd = np.ascontiguousarray(wd.reshape(HG, IB, P, 16, 512).astype(BF16))
    return wgu, wd


def _pack_x(x):
    tokens = np.ascontiguousarray(x, dtype=np.float32).reshape(N_CORES * T, H)
    xts = []
    for c in range(N_CORES):
        xs = tokens[c * T:(c + 1) * T]                       # [T, H]
        xt = xs.T.reshape(KO, P, T).transpose(1, 0, 2)        # [p, ko, t]
        xts.append(np.ascontiguousarray(xt.astype(BF16)))
    return xts


def _unpack_out(res_list, out_dtype):
    outs = []
    for c in range(N_CORES):
        a = res_list[c]["out"]                                # [HG, P, 4, T] f32
        ht = a.transpose(0, 2, 1, 3).reshape(H, T)            # [h, t]
        outs.append(ht.T)                                     # [t, h]
    full = np.concatenate(outs, axis=0)                       # [4096, H]
    return full.reshape(2, N_CORES * T // 2, H).astype(out_dtype, copy=False)


def _run(x, gate_w, up_w, down_w, gate_scale, up_scale, down_scale,
         trace=False, **run_kwargs):
    key = (float(gate_scale), float(up_scale), float(down_scale))
    if key not in _CACHE:
        _CACHE.clear()
        _CACHE[key] = _build_nc(*key)
    nc = _CACHE[key]

    wgu, wd = _pack_weights(gate_w, up_w, down_w)
    xts = _pack_x(x)
    in_maps = [{"xt": xts[c], "wgu": wgu, "wd": wd} for c in range(N_CORES)]
    try:
        res = run_bass_kernel_spmd(nc, in_maps, core_ids=list(range(N_CORES)),
                                   trace=trace, **run_kwargs)
    except Exception:
        # transient device/runtime hiccups: one retry
        res = run_bass_kernel_spmd(nc, in_maps, core_ids=list(range(N_CORES)),
                                   trace=trace, **run_kwargs)
    out = _unpack_out(res.results, np.asarray(x).dtype)
    return out, res


def kernel(x, gate_w, up_w, down_w, gate_scale, up_scale, down_scale):
    out, _ = _run(x, gate_w, up_w, down_w, gate_scale, up_scale, down_scale)
    return out


# revision 6
# speedup vs baseline: 1.0019x; 1.0019x over previous
"""BitNet MLP (SwiGLU, ternary weights) on 8 Trainium2 NeuronCores.

Strategy: 8-way data-parallel over the 4096 tokens (512 tokens/core),
weights replicated. No collectives. Everything is laid out
[feature, token] (transposed host-side) so every matmul has the
contraction dim on partitions and needs no on-device transposes:

  phase 2: gate/up projections + SwiGLU
      psum[i_tile(128), T=512] += wgu_tile[kp=128, 128].T @ xt[kp=128, T]
      inter = silu(gate*gs) * up                      (kept resident, bf16)
  phase 3: down projection
      psum[h_tile(128), T] += wd_tile[ip=128, 128].T @ inter[ip=128, T]
      out = psum * (us*ds)

All matmuls are bf16 (ternary weights are exact in bf16), N=512 = one
PSUM bank, K=128. Weights stream from HBM in 2 MiB blocks, alternating
between the two HWDGE rings (sync + scalar) so each ring runs at ~75
GB/s instead of one at 150. ~30 warm-up matmuls on a memset tile run
during the DMA lead-in so the PE's HAM clock-gate reaches 8/8 before
the first real matmul.
"""

import sys

for _p in ("/opt/trn_rl_repo",):
    if _p not in sys.path:
        sys.path.append(_p)

import numpy as np
import ml_dtypes

import concourse.bacc as bacc
import concourse.mybir as mybir
import concourse.tile as tile
from concourse.bass_utils import run_bass_kernel_spmd

BF16 = ml_dtypes.bfloat16

N_CORES = 8
H = 4096          # hidden
I = 11008         # intermediate
T = 512           # tokens per core (4096 / 8)
P = 128
KO = H // P       # 32 k-tiles for gate/up contraction
IT = I // P       # 86 i-tiles
NG2 = IT // 2     # 43 phase-2 groups (2 i-tiles each)
KH = 2            # k halves (16 k-tiles per DMA block)
HG = (H // P) // 4  # 8 phase-3 groups (4 h-tiles each)
IB = 6            # phase-3 i-blocks of 16 (86 padded to 96)
IT_PAD = IB * 16
N_WARM = 40       # PE warm-up matmuls (N=128 each) during DMA lead-in

_CACHE = {}


def _build_nc(gate_scale: float, up_scale: float, down_scale: float):
    nc = bacc.Bacc("TRN2", target_bir_lowering=False, debug=False,
                   enable_asserts=False, num_devices=N_CORES)
    f32 = mybir.dt.float32
    bf16 = mybir.dt.bfloat16

    xt_d = nc.dram_tensor("xt", [P, KO, T], bf16, kind="ExternalInput")
    wgu_d = nc.dram_tensor("wgu", [NG2, KH, P, 16, 512], bf16, kind="ExternalInput")
    wd_d = nc.dram_tensor("wd", [HG, IB, P, 16, 512], bf16, kind="ExternalInput")
    out_d = nc.dram_tensor("out", [HG, P, 4, T], f32, kind="ExternalOutput")

    blk_sizes = [16] * (IB - 1) + [IT - 16 * (IB - 1)]  # [16,16,16,16,16,6]

    with tile.TileContext(nc) as tc:
        with (
            tc.tile_pool(name="xpool", bufs=1) as xpool,
            tc.tile_pool(name="wpool", bufs=3) as wpool,
            tc.tile_pool(name="ipool", bufs=1) as ipool,
            tc.tile_pool(name="tpool", bufs=2) as tpool,
            tc.tile_pool(name="opool", bufs=2) as opool,
            tc.tile_pool(name="dpool", bufs=1) as dpool,
            tc.tile_pool(name="psum", bufs=2, space="PSUM") as psum,
        ):
            # ---- phase 0: PE warm-up ----
            # The HAM clock gate keeps the PE at 1.2 GHz until it has seen
            # ~3.4us of sustained matmul activity.  Spend the DMA lead-in
            # warming it on a memset tile so real matmuls run at 2.4 GHz
            # from the first one.
            dummy = dpool.tile([P, P], bf16)
            nc.vector.memset(dummy[:], 0)
            warm_ps = psum.tile([P, T], f32, tag="p0")
            for _ in range(N_WARM):
                nc.tensor.matmul(warm_ps[:, 0:P], dummy[:], dummy[:],
                                 start=True, stop=True)

            # xt goes on the scalar HWDGE ring (parallel to weight traffic
            # on the sync ring), in escalating k-slice chunks so the first
            # matmuls don't wait for the whole 4 MiB.
            xt_sb = xpool.tile([P, KO, T], bf16)
            for lo, hi in ((0, 1), (1, 2), (2, 3), (3, 4), (4, 6), (6, 8), (8, 10),
                           (10, 12), (12, 16), (16, 20), (20, 24), (24, 28), (28, 32)):
                nc.scalar.dma_start(xt_sb[:, lo:hi, :], xt_d.ap()[:, lo:hi, :])
            inter_sb = ipool.tile([P, IT, T], bf16)

            # ---- phase 2: gate/up matmuls + SwiGLU ----
            for g in range(NG2):
                pg0 = psum.tile([P, T], f32, tag="p0")
                pg1 = psum.tile([P, T], f32, tag="p1")
                pu0 = psum.tile([P, T], f32, tag="p2")
                pu1 = psum.tile([P, T], f32, tag="p3")
                for kh in range(KH):
                    w = wpool.tile([P, 16, 512], bf16, tag="w")
                    if g == 0 and kh == 0:
                        # escalating first blocks so MM 0 starts early; sized so
                        # arrival (~167 GB/s ring ramp) tracks warm consumption
                        for lo, hi in ((0, 1), (1, 2), (2, 3), (3, 4), (4, 6),
                                       (6, 8), (8, 10), (10, 13), (13, 16)):
                            nc.sync.dma_start(w[:, lo:hi, :],
                                              wgu_d.ap()[g, kh, :, lo:hi, :])
                    elif g == 0:
                        for lo, hi in ((0, 2), (2, 4), (4, 6), (6, 8), (8, 12), (12, 16)):
                            nc.sync.dma_start(w[:, lo:hi, :],
                                              wgu_d.ap()[g, kh, :, lo:hi, :])
                    else:
                        # alternate HWDGE rings: kh=0 on sync, kh=1 on scalar
                        eng = nc.sync if kh == 0 else nc.scalar
                        eng.dma_start(w[:], wgu_d.ap()[g, kh])
                    for ko in range(16):
                        k = kh * 16 + ko
                        rhs = xt_sb[:, k, :]
                        st, sp = k == 0, k == KO - 1
                        nc.tensor.matmul(pg0[:], w[:, ko, 0:128], rhs, start=st, stop=sp)
                        nc.tensor.matmul(pg1[:], w[:, ko, 128:256], rhs, start=st, stop=sp)
                        nc.tensor.matmul(pu0[:], w[:, ko, 256:384], rhs, start=st, stop=sp)
                        nc.tensor.matmul(pu1[:], w[:, ko, 384:512], rhs, start=st, stop=sp)
                for j, (pg, pu) in enumerate(((pg0, pu0), (pg1, pu1))):
                    silu_t = tpool.tile([P, T], bf16, tag="silu")
                    nc.scalar.activation(silu_t[:], pg[:],
                                         mybir.ActivationFunctionType.Silu,
                                         scale=gate_scale)
                    nc.vector.tensor_mul(inter_sb[:, 2 * g + j, :], silu_t[:], pu[:])

            # ---- phase 3: down matmul ----
            out_scale = up_scale * down_scale
            for hg in range(HG):
                pd = [psum.tile([P, T], f32, tag=f"p{j}", name=f"pd{j}") for j in range(4)]
                for b in range(IB):
                    wd_sb = wpool.tile([P, 16, 512], bf16, tag="w")
                    eng = nc.sync if b % 2 == 0 else nc.scalar
                    eng.dma_start(wd_sb[:], wd_d.ap()[hg, b])
                    # very last block of the kernel: finish the four psum
                    # chains j-sequentially so pd0 stops ~3.9us before pd3;
                    # each epilogue + out-DMA drains while later chains compute
                    stagger = (hg == HG - 1) and (b == IB - 1)
                    if stagger:
                        for j in range(4):
                            for io in range(blk_sizes[b]):
                                i = b * 16 + io
                                nc.tensor.matmul(pd[j][:],
                                                 wd_sb[:, io, j * 128:(j + 1) * 128],
                                                 inter_sb[:, i, :],
                                                 start=False, stop=(i == IT - 1))
                    else:
                        for io in range(blk_sizes[b]):
                            i = b * 16 + io
                            rhs = inter_sb[:, i, :]
                            st, sp = i == 0, i == IT - 1
                            for j in range(4):
                                nc.tensor.matmul(pd[j][:], wd_sb[:, io, j * 128:(j + 1) * 128],
                                                 rhs, start=st, stop=sp)
                ob = opool.tile([P, 4, T], f32, tag="ob")
                for j in range(4):
                    # alternate ACT/DVE (and the two HWDGE rings) so the final
                    # scale-copies and out-DMAs don't serialize on one queue
                    if j % 2 == 0:
                        nc.scalar.activation(ob[:, j, :], pd[j][:],
                                             mybir.ActivationFunctionType.Copy,
                                             scale=out_scale)
                        nc.sync.dma_start(out_d.ap()[hg, :, j, :], ob[:, j, :])
                    else:
                        nc.vector.tensor_scalar_mul(ob[:, j, :], pd[j][:], out_scale)
                        nc.scalar.dma_start(out_d.ap()[hg, :, j, :], ob[:, j, :])

    nc.compile()
    return nc


def _pack_weights(gate_w, up_w, down_w):
    # wgu[g, kh, p, ko, c]: c 0:128 -> gate i-tile 2g, 128:256 -> gate 2g+1,
    # 256:384 -> up 2g, 384:512 -> up 2g+1. p = k_p (contraction partition).
    def pack2(wm):
        a = np.ascontiguousarray(wm, dtype=np.float32).reshape(NG2, 256, KH, 16, P)
        return a.transpose(0, 2, 4, 3, 1)  # [g, kh, p, ko, m]

    gt = pack2(gate_w)
    ut = pack2(up_w)
    wgu = np.concatenate([gt, ut], axis=-1).astype(BF16)
    wgu = np.ascontiguousarray(wgu)

    # wd[hg, b, p, io, j*128+h_p] = down_w[(hg*4+j)*128+h_p, (b*16+io)*128+p]
    dp = np.zeros((H, IT_PAD * P), dtype=np.float32)
    dp[:, :I] = down_w
    d = dp.reshape(HG, 4, P, IB, 16, P)          # [hg, j, h_p, b, io, p]
    wd = d.transpose(0, 3, 5, 4, 1, 2)           # [hg, b, p, io, j, h_p]
    wd = np.ascontiguousarray(wd.reshape(HG, IB, P, 16, 512).astype(BF16))
    return wgu, wd


def _pack_x(x):
    tokens = np.ascontiguousarray(x, dtype=np.float32).reshape(N_CORES * T, H)
    xts = []
    for c in range(N_CORES):
        xs = tokens[c * T:(c + 1) * T]                       # [T, H]
        xt = xs.T.reshape(KO, P, T).transpose(1, 0, 2)        # [p, ko, t]
        xts.append(np.ascontiguousarray(xt.astype(BF16)))
    return xts


def _unpack_out(res_list, out_dtype):
    outs = []
    for c in range(N_CORES):
        a = res_list[c]["out"]                                # [HG, P, 4, T] f32
        ht = a.transpose(0, 2, 1, 3).reshape(H, T)            # [h, t]
        outs.append(ht.T)                                     # [t, h]
    full = np.concatenate(outs, axis=0)                       # [4096, H]
    return full.reshape(2, N_CORES * T // 2, H).astype(out_dtype, copy=False)


def _run(x, gate_w, up_w, down_w, gate_scale, up_scale, down_scale,
         trace=False, **run_kwargs):
    key = (float(gate_scale), float(up_scale), float(down_scale))
    if key not in _CACHE:
        _CACHE.clear()
        _CACHE[key] = _build_nc(*key)
    nc = _CACHE[key]

    wgu, wd = _pack_weights(gate_w, up_w, down_w)
    xts = _pack_x(x)
    in_maps = [{"xt": xts[c], "wgu": wgu, "wd": wd} for c in range(N_CORES)]
    try:
        res = run_bass_kernel_spmd(nc, in_maps, core_ids=list(range(N_CORES)),
                                   trace=trace, **run_kwargs)
    except Exception:
        # transient device/runtime hiccups: one retry
        res = run_bass_kernel_spmd(nc, in_maps, core_ids=list(range(N_CORES)),
                                   trace=trace, **run_kwargs)
    out = _unpack_out(res.results, np.asarray(x).dtype)
    return out, res


def kernel(x, gate_w, up_w, down_w, gate_scale, up_scale, down_scale):
    out, _ = _run(x, gate_w, up_w, down_w, gate_scale, up_scale, down_scale)
    return out
